# revision 3
# baseline (speedup 1.0000x reference)
import numpy as np

B, M, CNT = 16, 1024, 768
D, H, HD = 256, 8, 32
SCALE = HD ** -0.5


def _host_reference(features, Wq, Wk, Wv, Wout, b_out, gamma, beta,
                    W1, b1, W2, b2, slot, mask, topk):
    import jax
    import jax.numpy as jnp
    cpu = jax.devices('cpu')[0]
    ctx = jax.default_device(cpu)
    ctx.__enter__()
    features = jax.device_put(np.asarray(features), cpu)
    d = features.shape[-1]
    dense = jnp.zeros((B * M, d), features.dtype).at[jnp.asarray(slot)].set(features)
    dense = dense.reshape(B, M, d)

    def heads(x):
        return x.reshape(B, M, H, HD).transpose(0, 2, 1, 3)

    q = heads(dense @ Wq)
    k = heads(dense @ Wk)
    v = heads(dense @ Wv)

    dots = jnp.einsum('bhid,bhjd->bhij', q, k) * SCALE
    bias = -10000.0 * (1.0 - jnp.asarray(mask).astype(dots.dtype))
    dots = dots + bias[:, None, None, :]
    attn = jax.nn.softmax(dots, axis=-1)

    _, topk_idx = jax.lax.top_k(attn.sum(axis=1), int(topk))

    out = jnp.einsum('bhij,bhjd->bhid', attn, v)
    out = out.transpose(0, 2, 1, 3).reshape(B, M, H * HD)
    out = out @ jnp.asarray(Wout) + jnp.asarray(b_out)

    x = dense + out

    mu = x.mean(-1, keepdims=True)
    var = ((x - mu) ** 2).mean(-1, keepdims=True)
    xn = (x - mu) / jnp.sqrt(var + 1e-5) * jnp.asarray(gamma) + jnp.asarray(beta)
    h1 = jax.nn.gelu(xn @ jnp.asarray(W1) + jnp.asarray(b1), approximate=False)
    x = x + (h1 @ jnp.asarray(W2) + jnp.asarray(b2))

    feat_out = x.reshape(B * M, d)[jnp.asarray(slot)]
    res = np.asarray(feat_out), np.asarray(topk_idx, np.int32)
    ctx.__exit__(None, None, None)
    return res


def _device_kernel(features, Wq, Wk, Wv, Wout, b_out, gamma, beta,
                   W1, b1, W2, b2, slot, mask, topk):
    """Data-parallel over B on the 8 NeuronCores: each core computes the
    dense attention + MLP for 2 samples on its [CNT, D] block."""
    import jax
    import jax.numpy as jnp
    from functools import partial

    devs = [d for d in jax.devices() if 'cpu' not in str(d.platform).lower()]
    if len(devs) < 8:
        raise RuntimeError("need 8 neuron cores")
    devs = devs[:8]

    slot_np = np.asarray(slot)
    # scatter on host (pure data movement)
    dense = np.zeros((B * M, D), np.float32)
    dense[slot_np] = np.asarray(features, np.float32)
    dense = dense.reshape(B, M, D)
    # only rows < CNT are populated for the canonical slot layout; verify
    rows_ok = np.all(slot_np % M < CNT)
    X = dense[:, :CNT, :] if rows_ok else dense  # [B, CNT, D]
    n_rows = X.shape[1]

    mask_np = np.asarray(mask)
    valid_cols = mask_np[0].astype(bool)
    uniform_mask = np.all(mask_np == mask_np[0:1])
    Xv = X.reshape(8, B // 8, n_rows, D)

    @partial(jax.pmap, axis_name='i',
             in_axes=(0, None, None, None, None, None, None, None, None, None, None, None, None))
    def run(xb, Wq, Wk, Wv, Wout, b_out, gamma, beta, W1, b1, W2, b2, maskb):
        # xb: [B/8, n_rows, D]
        nb = xb.shape[0]

        def heads(t):
            return t.reshape(nb, n_rows, H, HD).transpose(0, 2, 1, 3)

        q = heads(xb @ Wq)
        k = heads(xb @ Wk)
        v = heads(xb @ Wv)
        dots = jnp.einsum('bhid,bhjd->bhij', q, k) * SCALE
        dots = dots + (-10000.0 * (1.0 - maskb))[None, None, None, :n_rows]
        attn = jax.nn.softmax(dots, axis=-1)
        scores = attn.sum(axis=1)                      # [nb, n_rows, n_rows]
        out = jnp.einsum('bhij,bhjd->bhid', attn, v)
        out = out.transpose(0, 2, 1, 3).reshape(nb, n_rows, D)
        out = out @ Wout + b_out
        x = xb + out
        mu = x.mean(-1, keepdims=True)
        var = ((x - mu) ** 2).mean(-1, keepdims=True)
        xn = (x - mu) / jnp.sqrt(var + 1e-5) * gamma + beta
        h1 = jax.nn.gelu(xn @ W1 + b1, approximate=False)
        x = x + (h1 @ W2 + b2)
        return x, scores

    args = [jnp.asarray(np.asarray(a, np.float32)) for a in
            (Wq, Wk, Wv, Wout, b_out, gamma, beta, W1, b1, W2, b2)]
    maskb = jnp.asarray(mask_np[0].astype(np.float32))
    y, scores = run(jnp.asarray(Xv), *args, maskb)
    y = np.asarray(y).reshape(B, n_rows, D)
    scores = np.asarray(scores).reshape(B, n_rows, n_rows)

    K = int(topk)
    # top-k on full [B, M, M] scores; cols >= CNT are exactly 0, rows >= CNT uniform
    scores_full = np.zeros((B, M, M), np.float32)
    scores_full[:, :n_rows, :n_rows] = scores
    if n_rows < M:
        scores_full[:, n_rows:, valid_cols] = float(H) / float(valid_cols.sum())
    import jax.numpy as jnp2
    _, topk_idx = jax.lax.top_k(jnp2.asarray(scores_full), K)
    topk_idx = np.asarray(topk_idx).astype(np.asarray(slot).dtype
                                           if np.issubdtype(np.asarray(slot).dtype, np.integer)
                                           else np.int32)

    # gather back
    x_full = np.zeros((B, M, D), np.float32)
    x_full[:, :n_rows, :] = y
    feat_out = x_full.reshape(B * M, D)[slot_np]
    if not rows_ok or not uniform_mask or not np.array_equal(
            valid_cols, np.arange(M) < CNT):
        raise RuntimeError("non-canonical structure")
    return feat_out.astype(np.float32), np.asarray(topk_idx, np.int32)


def kernel(**inputs):
    try:
        return _device_kernel(**inputs)
    except Exception:
        return _host_reference(**inputs)
